# revision 1
# baseline (speedup 1.0000x reference)
"""Trainium2 Bass kernel for nn_Capsule: capsule routing head.

Math: the einsum 'nco,pbo->bno' factorizes as xp[b,n,o] = W[n,o] * X[b,o]
with W = caps_weights.sum(c) (64x128) and X = x.sum(p) (256x128), so the
kernel is a memory-bound reduction of x (151 MB) followed by a tiny
per-batch routing loop (matmuls of size <= 128x64x128).

Sharding: data-parallel over batch (dim 1 of x), 32 batch elements per
core; caps_weights replicated; no cross-core communication.

Per-core pipeline:
  - 9 p-tiles of x (128, 4096) stream in via both HWDGE rings (sync +
    scalar engines), issued before everything else.
  - Reduction via fp32r matmuls with one-hot-column stationary matrices:
    for p-tile t and batch b, matmul(psum(32,128) +=
    E_b^T @ x_tile[:, b*128:(b+1)*128]) where E_b has ones in column b.
    All 288 matmuls accumulate into ONE psum bank; X lands directly as
    (32,128).  fp32r streams 1 col/cycle (4x faster than fp32); the
    stationary is exact 0/1, the moving operand is rounded (~1e-4 rel).
  - Routing in b-on-partitions layout: norms via DVE free-axis reduce,
    softmax over free axis, sqrt(q) computed as Exp(0.5*Ln(q)) so every
    activation lives in one ACT table (no mid-kernel table reloads; the
    table registry is pinned to 'natural_log_exp_and_others').
"""

import numpy as np

# ---- problem constants (hardcoded per contract) ----
P_TOT = 1152
BATCH = 256
O = 128
N_CAPS = 64
CAPS_DIM = 16
ITERATIONS = 3
N_CORES = 8
B_LOC = BATCH // N_CORES          # 32 batch elements per core
PT = P_TOT // 128                 # 9 p-tiles
FLAT = B_LOC * O                  # 4096 free elements per p-tile

_cache = {}


def _pin_act_table():
    """Force every ACT function onto the one table containing
    Exp+Ln+Square+Copy, so the kernel needs a single ACT_TABLE_LOAD."""
    import functools
    import concourse.hw_specs as hw_specs
    import concourse.bacc as bacc_mod

    if getattr(hw_specs.get_activation_tables, "_capsule_pinned", False):
        return
    orig = hw_specs.get_activation_tables

    @functools.cache
    def pinned(module_arch):
        tabs = orig(module_arch)
        keep = None
        for name, fns in tabs.items():
            names = {f.name for f in fns}
            if {"Exp", "Ln", "Square", "Copy", "Identity"} <= names:
                keep = name
                break
        if keep is None:
            return tabs
        return {n: (fns if n == keep else type(fns)()) for n, fns in tabs.items()}

    pinned._capsule_pinned = True
    hw_specs.get_activation_tables = pinned
    bacc_mod.get_activation_tables = pinned


def _build():
    _pin_act_table()
    import concourse.bacc as bacc
    import concourse.tile as tile
    import concourse.mybir as mybir
    from concourse.masks import make_identity

    f32 = mybir.dt.float32
    f32r = mybir.dt.float32r
    AX = mybir.AxisListType
    AF = mybir.ActivationFunctionType
    OP = mybir.AluOpType

    nc = bacc.Bacc(None, target_bir_lowering=False)

    # x declared f32r: same bytes as fp32, lets plain HWDGE DMAs feed the
    # fast fp32r matmul path with no cast.
    x_in = nc.dram_tensor("x", [P_TOT, B_LOC, O], f32r, kind="ExternalInput")
    w_in = nc.dram_tensor("caps_weights", [N_CAPS, CAPS_DIM, O], f32,
                          kind="ExternalInput")
    # one-hot stationary source: (128, 63) with ones in column 31, so
    # cst[:, 31-b : 63-b] is the one-hot-column-b matrix E_b.
    cst_in = nc.dram_tensor("cst", [128, 2 * B_LOC - 1], f32r,
                            kind="ExternalInput")
    out_d = nc.dram_tensor("out", [B_LOC, O], f32, kind="ExternalOutput")

    xv = x_in.rearrange("(t p) b o -> t p b o", p=128)   # (9, 128, 32, 128)

    NG = 3                      # slab groups
    GS = PT // NG               # slabs per group (3)
    GW = GS * O                 # moving width per batch (384)

    with tile.TileContext(nc) as tc:
        with (
            tc.tile_pool(name="xin", bufs=NG) as xpool,
            tc.tile_pool(name="wrk", bufs=1) as wrk,
            tc.tile_pool(name="small", bufs=1) as small,
            tc.tile_pool(name="ps", bufs=1, space="PSUM") as ps,
        ):
            # ---- DMAs first: x leads the sync ring; cst/w lead scalar ----
            zpat = small.tile([128, 2 * B_LOC - 1], f32r)
            nc.sync.dma_start(zpat[:], cst_in[:])
            w_sb = wrk.tile([N_CAPS, CAPS_DIM * O], f32)
            nc.scalar.dma_start(w_sb[:], w_in.rearrange("n c o -> n (c o)"))
            # group tiles hold 3 slabs in (b, s, o) layout so each batch's
            # matmul streams a contiguous 384-wide moving operand (fp32r
            # needs >=256 free for full rate)
            xgs = []
            for g in range(NG):
                xg = xpool.tile([128, B_LOC * GW], f32r, tag="xg",
                                name=f"xg{g}")
                xgs.append(xg)
            engs = [nc.sync, nc.scalar]
            xg_vs = [xgs[g][:].rearrange("p (s b o) -> p s b o",
                                         b=B_LOC, s=GS) for g in range(NG)]
            # natural slab order alternating rings; the last group's slabs
            # 7 and 8 are batch-split so after the stream ends only one
            # batch-range (8 matmuls) of reduction work remains.
            for t in range(PT - 2):
                g, sidx = divmod(t, GS)
                engs[t % 2].dma_start(xg_vs[g][:, sidx, :, :], xv[t])
            for t in (PT - 2, PT - 1):
                g, sidx = divmod(t, GS)
                for dsub in range(4):
                    b0 = dsub * (B_LOC // 4)
                    b1 = b0 + B_LOC // 4
                    engs[(t + dsub) % 2].dma_start(
                        xg_vs[g][:, sidx, b0:b1, :], xv[t][:, b0:b1, :])

            # ---- capsule weight prep (overlaps the x stream) ----
            t1 = wrk.tile([N_CAPS, 8 * O], f32)
            nc.vector.tensor_tensor(t1[:], w_sb[:, :8 * O], w_sb[:, 8 * O:], OP.add)
            t2 = wrk.tile([N_CAPS, 4 * O], f32)
            nc.vector.tensor_tensor(t2[:], t1[:, :4 * O], t1[:, 4 * O:], OP.add)
            t3 = wrk.tile([N_CAPS, 2 * O], f32)
            nc.vector.tensor_tensor(t3[:], t2[:, :2 * O], t2[:, 2 * O:], OP.add)
            w_no = wrk.tile([N_CAPS, O], f32)          # W[n,o]
            nc.vector.tensor_tensor(w_no[:], t3[:, :O], t3[:, O:], OP.add)

            ident = small.tile([128, 128], f32)
            make_identity(nc, ident[:])

            ps_wt = ps.tile([O, N_CAPS], f32, tag="ps_wt")
            nc.tensor.transpose(ps_wt[:], w_no[:], ident[:N_CAPS, :N_CAPS])
            wt_on = wrk.tile([O, N_CAPS], f32)          # W^T[o,n]
            nc.vector.tensor_copy(wt_on[:], ps_wt[:])
            # S0[b,o] = (1/64) sum_n W[n,o] for every b (uniform coeffs0)
            unif = small.tile([N_CAPS, B_LOC], f32)
            nc.vector.memset(unif[:], 1.0 / N_CAPS)
            ps_s0 = ps.tile([B_LOC, O], f32, tag="ps_s")
            nc.tensor.matmul(ps_s0[:], unif[:], w_no[:], start=True, stop=True)

            # ---- reduction: X[b,o] = sum_p x[p,b,o] ----
            # per group: 32 matmuls (one per batch) with one-hot-column
            # stationary; moving is a strided (s,o) view (3 chunks of 128)
            # so out free = 384 >= 256 keeps fp32r at full rate while the
            # slab DMAs stay fully contiguous. psum[b, s*128+o] holds
            # per-slab partial sums; combine with strided reduces.
            # all 96 matmuls accumulate into ONE psum tile: the (b, s*128+o)
            # sub-column layout is identical across groups, so each group
            # keeps adding onto the same partial sums; one strided reduce
            # then combines the three per-slab sub-columns.
            ps_x = ps.tile([B_LOC, GW], f32, tag="ps_x")
            ps_dmy = ps.tile([B_LOC, 256], f32, tag="ps_wt")
            for g in range(NG):
                mv = xgs[g][:].rearrange("p (s b o) -> p b s o",
                                         b=B_LOC, s=GS)
                if g == NG - 1:
                    # warm-keeper bridge: hold the PE HAM un-throttled
                    # across the gap before the final group; paced by
                    # slab-6 data (arrives mid-stream), results unread.
                    for k in range(10):
                        nc.tensor.matmul(
                            ps_dmy[:], zpat[:, B_LOC - 1: 2 * B_LOC - 1],
                            xgs[g][:, k * 384: k * 384 + 256],
                            start=True, stop=True, skip_group_check=True)
                for b in range(B_LOC):
                    nc.tensor.matmul(
                        ps_x[:], zpat[:, B_LOC - 1 - b: 2 * B_LOC - 1 - b],
                        mv[:, b, :, :],
                        start=(g == 0 and b == 0),
                        stop=(g == NG - 1 and b == B_LOC - 1),
                        skip_group_check=True)
            x32 = wrk.tile([B_LOC, O], f32)             # X[b,o]
            nc.vector.tensor_reduce(
                x32[:], ps_x[:].rearrange("p (s o) -> p o s", s=GS),
                AX.X, OP.add)

            # ---- routing (b on partitions) ----
            u = wrk.tile([B_LOC, O], f32)
            sq = wrk.tile([B_LOC, O], f32)
            nsq = wrk.tile([B_LOC, 1], f32)
            lnq = wrk.tile([B_LOC, 1], f32)
            norm = wrk.tile([B_LOC, 1], f32)
            den = wrk.tile([B_LOC, 1], f32)
            rden = wrk.tile([B_LOC, 1], f32)
            scale = wrk.tile([B_LOC, 1], f32)

            for it in range(ITERATIONS):
                if it == 0:
                    nc.vector.tensor_tensor(u[:], x32[:], ps_s0[:], OP.mult)
                else:
                    # S_e[b,o] = sum_n exT[n,b] W[n,o]; u = X*S_e*rsum
                    ps_s = ps.tile([B_LOC, O], f32, tag="ps_s", name="ps_s")
                    nc.tensor.matmul(ps_s[:], exT[:], w_no[:],
                                     start=True, stop=True)
                    ue = wrk.tile([B_LOC, O], f32, tag="ue")
                    nc.vector.tensor_tensor(ue[:], x32[:], ps_s[:], OP.mult)
                    nc.vector.tensor_scalar_mul(u[:], ue[:], rsum[:])
                # nsq = sum_o u^2 (free-axis) on DVE
                nc.vector.tensor_tensor(sq[:], u[:], u[:], OP.mult)
                nc.vector.tensor_reduce(nsq[:], sq[:], AX.X, OP.add)
                nc.vector.tensor_scalar_add(den[:], nsq[:], 1.0)
                if it < ITERATIONS - 1:
                    ux = wrk.tile([B_LOC, O], f32, tag="ux")
                    nc.vector.tensor_tensor(ux[:], u[:], x32[:], OP.mult)
                # scale = sqrt(q)/(1+q); sqrt(q) = Exp(0.5*Ln(q)); the DVE
                # reciprocal of (1+q) overlaps the two ACT table lookups
                nc.scalar.activation(lnq[:], nsq[:], AF.Ln)
                nc.scalar.activation(norm[:], lnq[:], AF.Exp, scale=0.5)
                nc.vector.reciprocal(rden[:], den[:])
                nc.vector.tensor_tensor(scale[:], norm[:], rden[:], OP.mult)

                if it < ITERATIONS - 1:
                    # t = routed*X = scale*u*X ; delta[b,n] = sum_o t W^T
                    tb = wrk.tile([B_LOC, O], f32, tag="tb")
                    nc.vector.tensor_scalar_mul(tb[:], ux[:], scale[:])
                    ps_t = ps.tile([O, B_LOC], f32, tag="ps_t")
                    nc.tensor.transpose(ps_t[:], tb[:], ident[:B_LOC, :B_LOC])
                    tT = wrk.tile([O, B_LOC], f32, tag="tT")
                    nc.vector.tensor_copy(tT[:], ps_t[:])
                    ps_d = ps.tile([B_LOC, N_CAPS], f32, tag="ps_d")
                    nc.tensor.matmul(ps_d[:], tT[:], wt_on[:],
                                     start=True, stop=True)
                    # softmax over n (free axis, logits O(10): exp-safe);
                    # normalization deferred through rsum
                    ex = wrk.tile([B_LOC, N_CAPS], f32, tag="ex")
                    ssum = wrk.tile([B_LOC, 1], f32, tag="ssum")
                    if it == 0:
                        nc.scalar.activation(ex[:], ps_d[:], AF.Exp,
                                             accum_out=ssum[:])
                        logits = wrk.tile([B_LOC, N_CAPS], f32, tag="lg")
                        nc.vector.tensor_copy(logits[:], ps_d[:])
                    else:
                        lg2 = wrk.tile([B_LOC, N_CAPS], f32, tag="lg2")
                        nc.vector.tensor_tensor(lg2[:], logits[:], ps_d[:],
                                                OP.add)
                        nc.scalar.activation(ex[:], lg2[:], AF.Exp,
                                             accum_out=ssum[:])
                    rsum = wrk.tile([B_LOC, 1], f32, tag="rsum")
                    nc.vector.reciprocal(rsum[:], ssum[:])
                    ps_ct = ps.tile([N_CAPS, B_LOC], f32, tag="ps_ct")
                    nc.tensor.transpose(ps_ct[:], ex[:],
                                        ident[:B_LOC, :B_LOC])
                    exT = wrk.tile([N_CAPS, B_LOC], f32, tag="exT")
                    nc.vector.tensor_copy(exT[:], ps_ct[:])
                else:
                    out_sb = wrk.tile([B_LOC, O], f32, tag="out_sb")
                    nc.vector.tensor_scalar_mul(out_sb[:], u[:], scale[:])
                    nc.sync.dma_start(out_d[:], out_sb[:])

    nc.compile()
    return nc


def run_with_results(x: np.ndarray, caps_weights: np.ndarray, **run_kwargs):
    """Run the SPMD kernel; returns (output (256,1,128), BassKernelResults)."""
    from concourse.bass_utils import run_bass_kernel_spmd

    if "nc" not in _cache:
        _cache["nc"] = _build()
    nc = _cache["nc"]

    x = np.ascontiguousarray(x, dtype=np.float32)
    caps_weights = np.ascontiguousarray(caps_weights, dtype=np.float32)
    cst = np.zeros((128, 2 * B_LOC - 1), dtype=np.float32)
    cst[:, B_LOC - 1] = 1.0

    in_maps = []
    for c in range(N_CORES):
        in_maps.append({
            "x": np.ascontiguousarray(x[:, c * B_LOC:(c + 1) * B_LOC, :]),
            "caps_weights": caps_weights,
            "cst": cst,
        })
    res = run_bass_kernel_spmd(nc, in_maps, core_ids=list(range(N_CORES)),
                               **run_kwargs)
    out = np.concatenate([res.results[c]["out"] for c in range(N_CORES)], axis=0)
    return out.reshape(BATCH, 1, O), res


def kernel(x: np.ndarray, caps_weights: np.ndarray) -> np.ndarray:
    out, _ = run_with_results(x, caps_weights)
    return out

